# revision 35
# baseline (speedup 1.0000x reference)
"""Trainium2 Bass kernel for a 2-layer GRU decoder with teacher forcing.

Problem (hardcoded shapes):
  B=2048, T=32, H=256, V=128, L=2, 8 NeuronCores, data-parallel on batch.

Math per core (B_local=256), all activations kept TRANSPOSED [feature, batch]:
  tokens -> one-hot [V, B]  (host precomputed)
  layer-1 input projection:  xi1 = table @ onehot, where
      table[v,:] = relu(embed_w)[v] @ W_ih[0].T + b_ih[0] (+ b_hh[0] on r,z rows)
  per step t (both layers):  psum_rz = xi_rz + gh_rz (+biases), sigmoid -> r,z
      rhn = (gh_n + b_hhn) * r ; npre = xi_n (+b_ihn) + rhn ; n = tanh(npre)
      h = n + z*(h_prev - n)
  Layer-2 h states accumulate into one big SBUF buffer; fc + log_softmax run
  as a batched tail phase (limits ACT table-set switches; exp and ln phases
  are separated because walrus assigns them different table sets):
      out = (logits + fc_b) - ln(sum_v exp(logits + fc_b)); the logit range
      here is tiny, so no max-subtraction is needed for a stable exp.
  Matmul operands (weights, one-hot, h states) are bf16 (1 cycle/row on the
  PE vs 4 for fp32); all accumulation stays fp32 in PSUM, gate arithmetic
  runs on fp32 PSUM operands, and the softmax/output path is fp32.
  L2 is software-pipelined one step behind L1 so the in-order PE never waits
  on a gate chain; the post-sigmoid pipeline is split per 128-row half so
  next-step matmuls start as soon as their half of h lands.
"""

import ml_dtypes
import numpy as np

BF16 = ml_dtypes.bfloat16

B, T, H, V, L = 2048, 32, 256, 128, 2
G = 3 * H          # 768 gate rows
NCORES = 8
Bl = B // NCORES   # 256
KC = 2             # K chunks of 128 for H=256 contractions

_CACHE = {}


def _build_program():
    from contextlib import ExitStack

    import concourse.bacc as bacc
    import concourse.tile as tile
    import concourse.mybir as mybir

    f32 = mybir.dt.float32
    wd = mybir.dt.bfloat16   # matmul-operand / gate-value compute dtype
    AF = mybir.ActivationFunctionType
    OP = mybir.AluOpType

    nc = bacc.Bacc("TRN2", target_bir_lowering=False, debug=False)

    def rr(ap):
        return ap

    din = {}
    for name, shape in [
        ("table", (V, G)),
        ("whh0", (128, KC, G)),
        ("whh1", (128, KC, G)),
        ("wih1", (128, KC, G)),
        ("fcw", (128, KC, V)),
        ("fcb", (1, V)),
        ("bias_c", (128, 6)),
        ("bias_r", (1, 512)),
        ("h0", (L, 128, KC, Bl)),
        ("oh", (T, V, Bl)),
        ("eye", (128, 128)),
    ]:
        dt_in = f32 if name == "bias_c" else wd
        din[name] = nc.dram_tensor(name, shape, dt_in, kind="ExternalInput")
    d_logits = nc.dram_tensor("logits", (Bl, T, V), f32, kind="ExternalOutput")
    d_hidden = nc.dram_tensor("hidden", (L, Bl, H), f32, kind="ExternalOutput")

    with tile.TileContext(nc) as tc, ExitStack() as ctx:
        consts = ctx.enter_context(tc.tile_pool(name="consts", bufs=1))
        hpool = ctx.enter_context(tc.tile_pool(name="hpool", bufs=5))
        work = ctx.enter_context(tc.tile_pool(name="work", bufs=5))
        ohpool = ctx.enter_context(tc.tile_pool(name="ohpool", bufs=6))
        bigp = ctx.enter_context(tc.tile_pool(name="bigp", bufs=1))
        prz = ctx.enter_context(tc.tile_pool(name="prz", bufs=2, space="PSUM"))
        pghn = ctx.enter_context(tc.tile_pool(name="pghn", bufs=1, space="PSUM"))
        paux = ctx.enter_context(tc.tile_pool(name="paux", bufs=2, space="PSUM"))

        # ---- constants into SBUF
        csb = {}
        for name, shape in [
            ("table", [V, G]),
            ("whh0", [128, KC, G]),
            ("whh1", [128, KC, G]),
            ("wih1", [128, KC, G]),
            ("fcw", [128, KC, V]),
            ("fcb", [1, V]),
            ("bias_c", [128, 6]),
            ("bias_r", [1, 512]),
            ("eye", [128, 128]),
        ]:
            t_ = consts.tile(shape, f32 if name == "bias_c" else wd,
                             name=f"c_{name}", tag=f"c_{name}")
            nc.sync.dma_start(out=t_[:], in_=din[name][:])
            csb[name] = t_
        ones_sb = consts.tile([1, Bl], wd, name="ones_sb", tag="ones_sb")
        nc.vector.memset(ones_sb[:], 1.0)

        # fc_b broadcast to all 128 partitions (K=1 matmul + copy)
        p_b = paux.tile([128, 2, Bl], f32, name="p_b", tag="paux")
        nc.tensor.matmul(p_b[:, 0, 0:V], ones_sb[0:1, 0:128], csb["fcb"][0:1, :],
                         start=True, stop=True)
        fcb_bc = consts.tile([128, V], f32, name="fcb_bc", tag="fcb_bc")
        nc.scalar.activation(fcb_bc[:], p_b[:, 0, 0:V], AF.Copy)

        # layer-2 hidden states for the whole sequence (written in place by
        # the combine; read by the tail fc phase). index 0 = h0.
        h2_seq = bigp.tile([128, T + 1, KC, Bl], wd, name="h2_seq", tag="h2_seq")
        s_all = bigp.tile([128, 2, T], f32, name="s_all", tag="s_all")
        c_all = bigp.tile([128, 2, T], f32, name="c_all", tag="c_all")
        out_sb = bigp.tile([128, 2, T, V], f32, name="out_sb", tag="out_sb")

        whh = [csb["whh0"], csb["whh1"]]
        table = csb["table"]
        wih1 = csb["wih1"]

        h1 = hpool.tile([128, KC, Bl], wd, name="h1", tag="h1")
        nc.sync.dma_start(out=h1[:], in_=din["h0"][0])
        nc.sync.dma_start(out=h2_seq[:, 0, :, :], in_=din["h0"][1])
        h_cur = [h1, h2_seq[:, 0, :, :]]

        # ---------------- main recurrence ----------------
        # Software pipeline: L2 is emitted one step behind L1 so that every
        # matmul's inputs are already computed when PE (in-order) reaches it.
        h1_states = [h_cur[0]]
        h2_states = [h_cur[1]]

        def emit_layer(lay, t, oh_t):
            h_in = h1_states[t] if lay == 0 else h2_states[t]
            p_rz = prz.tile([128, 4, Bl], f32, name="p_rz", tag="p_rz")
            p_ghn = pghn.tile([128, 2, Bl], f32, name="p_ghn", tag="p_ghn")
            p_xin = paux.tile([128, 2, Bl], f32, name="p_xin", tag="paux")

            # PSUM zero-region rule: one start..stop group per 2KB bank.
            if lay == 0:
                for m in range(4):
                    nc.tensor.matmul(
                        p_rz[:, m, :], rr(table[:, m * 128 : (m + 1) * 128]),
                        rr(oh_t[:]), start=(m % 2 == 0), stop=False)
                for m in (4, 5):
                    nc.tensor.matmul(
                        p_xin[:, m - 4, :], rr(table[:, m * 128 : (m + 1) * 128]),
                        rr(oh_t[:]), start=(m == 4), stop=(m == 5))
            else:
                for m in range(4):
                    nc.tensor.matmul(
                        p_rz[:, m, :],
                        rr(csb["bias_r"][0:1, m * 128 : (m + 1) * 128]),
                        rr(ones_sb[0:1, :]), start=(m % 2 == 0), stop=False)
                x2 = h1_states[t + 1]
                for m in range(4):
                    for k in range(KC):
                        nc.tensor.matmul(
                            p_rz[:, m, :], rr(wih1[:, k, m * 128 : (m + 1) * 128]),
                            rr(x2[:, k, :]), start=False, stop=False)
                for m in (4, 5):
                    for k in range(KC):
                        nc.tensor.matmul(
                            p_xin[:, m - 4, :],
                            rr(wih1[:, k, m * 128 : (m + 1) * 128]),
                            rr(x2[:, k, :]), start=(m == 4 and k == 0),
                            stop=False)

            for m in range(4):
                for k in range(KC):
                    nc.tensor.matmul(
                        p_rz[:, m, :], rr(whh[lay][:, k, m * 128 : (m + 1) * 128]),
                        rr(h_in[:, k, :]), start=False,
                        stop=(m % 2 == 1 and k == 1))
            for m in (4, 5):
                for k in range(KC):
                    nc.tensor.matmul(
                        p_ghn[:, m - 4, :],
                        rr(whh[lay][:, k, m * 128 : (m + 1) * 128]),
                        rr(h_in[:, k, :]), start=(m == 4 and k == 0),
                        stop=(m == 5 and k == 1))

            # r-sigmoid first (on the critical chain), z-sigmoid after
            # (feeds only the off-chain z'/zh helpers).
            rz = work.tile([128, 4, Bl], wd, name="rz", tag="rz")
            nc.scalar.activation(rz[:, 0:2, :], p_rz[:, 0:2, :], AF.Sigmoid)
            nc.scalar.activation(rz[:, 2:4, :], p_rz[:, 2:4, :], AF.Sigmoid)
            zp = work.tile([128, 2, Bl], wd, name="zp", tag="zp")
            nc.vector.tensor_scalar(zp[:], rz[:, 2:4, :], -1.0, 1.0,
                                    op0=OP.mult, op1=OP.add)
            zh = work.tile([128, 2, Bl], wd, name="zh", tag="zh")
            nc.gpsimd.tensor_tensor(zh[:], rz[:, 2:4, :], h_in[:], op=OP.mult)

            bc = csb["bias_c"]
            boff = 0 if lay == 0 else 2
            # k-split pipeline: each half proceeds independently so next-step
            # matmuls on half k can start as soon as h_new[:, k] lands.
            rhn = work.tile([128, 2, Bl], wd, name="rhn", tag="rhn")
            npre = (work.tile([128, 2, Bl], wd, name="npre", tag="npre")
                    if lay == 0 else None)
            nn_ = work.tile([128, 2, Bl], wd, name="nn_", tag="nn_")
            u = work.tile([128, 2, Bl], wd, name="u", tag="u")
            if lay == 0:
                h_new = hpool.tile([128, KC, Bl], wd, name="h1", tag="h1")
                h_dst = h_new[:]
            else:
                h_dst = h2_seq[:, t + 1, :, :]
            for k in range(2):
                nc.vector.scalar_tensor_tensor(
                    rhn[:, k, :], p_ghn[:, k, :], bc[:, boff + k : boff + k + 1],
                    rz[:, k, :], op0=OP.add, op1=OP.mult)
            if lay == 0:
                for k in range(2):
                    nc.vector.tensor_tensor(npre[:, k, :], rhn[:, k, :],
                                            p_xin[:, k, :], op=OP.add)
                    nc.scalar.activation(nn_[:, k, :], npre[:, k, :], AF.Tanh)
                    nc.vector.tensor_tensor(u[:, k, :], zp[:, k, :], nn_[:, k, :],
                                            op=OP.mult)
                    nc.vector.tensor_tensor(h_new[:, k, :],
                                            u[:, k, :], zh[:, k, :], op=OP.add)
                h1_states.append(h_new[:])
                return None

            h2_states.append(h_dst)

            def finish():
                # rhn folded into the xi_n PSUM bank by an identity matmul
                # (PE has slack; saves a psum-bound DVE add per half), then
                # tanh reads PSUM directly.
                for k in range(2):
                    nc.tensor.matmul(
                        p_xin[:, k, :], csb["eye"][:], rhn[:, k, :],
                        start=False, stop=(k == 1))
                for k in range(2):
                    nc.scalar.activation(nn_[:, k, :], p_xin[:, k, :], AF.Tanh,
                                         bias=bc[:, 4 + k : 5 + k])
                    nc.vector.tensor_tensor(u[:, k, :], zp[:, k, :], nn_[:, k, :],
                                            op=OP.mult)
                    nc.vector.tensor_tensor(h_dst[:, k, :],
                                            u[:, k, :], zh[:, k, :], op=OP.add)
            return finish

        logits_sb = bigp.tile([128, 2, T, V], f32, name="logits_sb",
                              tag="logits_sb")

        def emit_fc(s):
            p_fc = pghn.tile([128, 2, V], f32, name="p_fc", tag="p_fc")
            for mb in range(2):
                nc.tensor.matmul(
                    p_fc[:, mb, :], ones_sb[0:1, 0:128], csb["fcb"][0:1, :],
                    start=(mb == 0), stop=False)
            for mb in range(2):
                for k in range(KC):
                    nc.tensor.matmul(
                        p_fc[:, mb, :],
                        h2_seq[:, s + 1, k, mb * 128 : (mb + 1) * 128],
                        csb["fcw"][:, k, :], start=False,
                        stop=(mb == 1 and k == 1))
            nc.scalar.activation(logits_sb[:, 0, s, :], p_fc[:, 0, :], AF.Copy)
            nc.vector.tensor_copy(logits_sb[:, 1, s, :], p_fc[:, 1, :])

        for t in range(T):
            oh_t = ohpool.tile([V, Bl], wd, name="oh_t", tag="oh_t")
            nc.sync.dma_start(out=oh_t[:], in_=din["oh"][t])
            emit_layer(0, t, oh_t)
            fin = emit_layer(1, t - 1, None) if t >= 1 else None
            if t >= 2:
                emit_fc(t - 2)
            if fin is not None:
                fin()
        fin = emit_layer(1, T - 1, None)
        fin()
        for s in range(T - 2, T):
            emit_fc(s)
        h_cur = [h1_states[-1], h2_states[-1]]

        # ---------------- tail: fc + log_softmax + hidden transpose -------
        # zero "gate" that depends on the final h2 -> keeps every Exp after
        # the recurrence so the ACT table set switches exactly once.
        gate = consts.tile([128, 1], f32, name="gate", tag="gate")
        nc.vector.tensor_scalar_mul(gate[:], h2_seq[:, T, 0, 0:1], 0.0)

        # hidden: transpose [H,B] -> [B,H] via PE, 128x128 blocks
        for lay in range(L):
            for k in range(KC):
                for mb in range(2):
                    pt = paux.tile([128, 2, Bl], wd, name="pt", tag="paux")
                    nc.tensor.matmul(
                        pt[:, 0, 0:128],
                        h_cur[lay][:, k, mb * 128 : (mb + 1) * 128], csb["eye"][:],
                        is_transpose=True, start=True, stop=True)
                    htr = work.tile([128, 128], f32, name="htr", tag="htr")
                    nc.vector.tensor_copy(htr[:], pt[:, 0, 0:128])
                    nc.sync.dma_start(
                        out=d_hidden[lay, mb * 128 : (mb + 1) * 128,
                                     k * 128 : (k + 1) * 128],
                        in_=htr[:])

        # exp (one table switch) in big per-octet ops; V-sums via DVE reduce
        # (bf16 e-values: ~3e-4 relative on the sums, negligible here)
        for t0 in range(0, T, 8):
            for mb in range(2):
                e_oct = work.tile([128, 8, V], wd, name="e_oct", tag="e_oct")
                nc.scalar.activation(
                    e_oct[:], logits_sb[:, mb, t0 : t0 + 8, :], AF.Exp,
                    bias=gate[:, 0:1])
                nc.vector.tensor_reduce(
                    s_all[:, mb, t0 : t0 + 8], e_oct[:],
                    axis=mybir.AxisListType.X, op=OP.add)
            for mb in range(2):
                nc.scalar.activation(
                    c_all[:, mb, t0 : t0 + 8], s_all[:, mb, t0 : t0 + 8], AF.Ln)
                for t in range(t0, t0 + 8):
                    nc.vector.tensor_single_scalar(
                        out_sb[:, mb, t, :], logits_sb[:, mb, t, :],
                        c_all[:, mb, t : t + 1], op=OP.subtract)
                nc.sync.dma_start(
                    out=d_logits[mb * 128 : (mb + 1) * 128, t0 : t0 + 8, :],
                    in_=out_sb[:, mb, t0 : t0 + 8, :])


    nc.compile()
    return nc


def _pack_host(inputs):
    enc_hidden = np.ascontiguousarray(np.asarray(inputs["enc_hidden"], np.float32))
    tgt = np.asarray(inputs["tgt_tensor"])
    embed_w = np.asarray(inputs["embed_w"], np.float32)
    W_ih = np.asarray(inputs["W_ih"], np.float32)
    W_hh = np.asarray(inputs["W_hh"], np.float32)
    b_ih = np.asarray(inputs["b_ih"], np.float32)
    b_hh = np.asarray(inputs["b_hh"], np.float32)
    fc_w = np.asarray(inputs["fc_w"], np.float32)
    fc_b = np.asarray(inputs["fc_b"], np.float32)

    er = np.maximum(embed_w, 0.0)
    table = er @ W_ih[0].T + b_ih[0][None, :]
    table[:, 0:512] += b_hh[0][None, 0:512]
    table = np.ascontiguousarray(table, BF16)

    def packT(w):  # [G,H] -> lhsT chunks [128, KC, G]
        return np.ascontiguousarray(
            w.T.reshape(KC, 128, -1).transpose(1, 0, 2), BF16)

    shared = dict(
        table=table,
        whh0=packT(W_hh[0]), whh1=packT(W_hh[1]), wih1=packT(W_ih[1]),
        fcw=packT(fc_w),
        fcb=np.ascontiguousarray(fc_b.reshape(1, V), BF16),
        bias_c=np.ascontiguousarray(
            np.stack([b_hh[0][512:640], b_hh[0][640:768],
                      b_hh[1][512:640], b_hh[1][640:768],
                      b_ih[1][512:640], b_ih[1][640:768]], axis=1)),
        bias_r=np.ascontiguousarray(
            (b_ih[1] + b_hh[1])[:512].reshape(1, 512), BF16),
        eye=np.eye(128, dtype=BF16),
    )

    inp_tok = np.concatenate(
        [np.zeros((B, 1), tgt.dtype), tgt[:, : T - 1]], axis=1)

    in_maps = []
    for c in range(NCORES):
        sl = slice(c * Bl, (c + 1) * Bl)
        toks = inp_tok[sl].astype(np.int32)  # [Bl, T]
        oh = (np.arange(V, dtype=np.int32)[None, :, None]
              == toks.T[:, None, :]).astype(BF16)  # [T, V, Bl]
        h0 = enc_hidden[:, sl, :].transpose(0, 2, 1)  # [L, H, Bl]
        h0p = np.ascontiguousarray(
            h0.reshape(L, KC, 128, Bl).transpose(0, 2, 1, 3), BF16)
        in_maps.append(dict(shared, oh=np.ascontiguousarray(oh), h0=h0p))
    return in_maps


def _get_nc():
    if "nc" not in _CACHE:
        _CACHE["nc"] = _build_program()
    return _CACHE["nc"]


def kernel(**inputs):
    from concourse.bass_utils import run_bass_kernel_spmd

    nc = _get_nc()
    in_maps = _pack_host(inputs)
    res = run_bass_kernel_spmd(nc, in_maps, core_ids=list(range(NCORES)))

    logits = np.empty((B, T, V), np.float32)
    hidden = np.empty((L, B, H), np.float32)
    for c in range(NCORES):
        sl = slice(c * Bl, (c + 1) * Bl)
        logits[sl] = res.results[c]["logits"]
        hidden[:, sl, :] = res.results[c]["hidden"]
    return logits, hidden
